# revision 46
# baseline (speedup 1.0000x reference)
"""CenterLossA on 8 Trainium2 NeuronCores — fp8-streamed, bf16 compute.

loss = main * (1 + 1/distocen) / 2 / B, where
  main     = sum_i ||f_i - c_{l_i}||^2
  distocen = sum_i sum_{k != l_i} ||f_i - c_k||^2

Algebraic reduction (everything needed from feat in ONE pass):
  main  = S_ff - 2*T1 + C1
  total = sum_i sum_k ||f_i - c_k||^2 = 3*S_ff - 2*T_all + B*Cn
  distocen = total - main
with
  S_ff  = sum(feat^2)                      (scalar)
  s_k   = sum_{i: l_i = k} f_i             ([3, D] per-class column sums)
  T1    = sum_k s_k . c_k ;  T_all = (sum_k s_k) . (sum_k c_k)
  C1    = sum_k n_k ||c_k||^2 ;  Cn = sum_k ||c_k||^2 ; n_k = count(label==k)

This is memory-regime: the f32 version sits exactly on the per-core HBM
streaming floor (~96-98 us for 32 MiB at ~350 GB/s/core; DMA-only programs
measure the same), so the only levers are BYTES and then engine throughput:
  f32 stream                 : ~92-98 us/pass  (HBM-bound, 32 MiB)
  bf16 stream                : ~48 us/pass     (HBM-bound, 16 MiB)
  fp8 stream, all cast->bf16 : ~42 us/pass     (SBUF-fabric-bound: 16 MiB of
                                                bf16 written at ~407 GB/s)
  fp8 mixed residency        : ~39 us/pass     (ACT-bound: ACT's 5/8 share
                                                stays fp8 in SBUF, only
                                                DVE's 3/8 is cast to bf16)
Engine poles at the final split: ACT 35.8, PE 34.1 (27.3 one-hot matmuls +
6.8 square-chunk reduce), DMA ~30 single-queue, DVE 22. All feat DMAs ride
ONE SWDGE queue: a second concurrent queue (sync HWDGE fp8 + gpsimd cast)
was measured to serialize against it and add ~7 us/pass of lost overlap.
ACT squares fp8 at full 1x rate (measured; no fp8 penalty).

Numerics: host casts feat f32 -> e4m3 (RN); the SWDGE DMA upcasts e4m3 ->
bf16 EXACTLY. e4m3 quantization biases S_ff by E[delta^2] ~ 1e-3; measured
end-to-end loss error 3.7e-4 vs the f32 reference — ~50x inside the 2e-2
gate. All accumulation is f32 on-chip / f64 on host.

Device kernel (data-parallel over batch, 4096 rows/core), one stream:
  - SWDGE casting DMA: 4 supertiles of [128, 8x2048] (2 MiB fp8 HBM-side ->
    4 MiB bf16 SBUF-side), triple-buffered
  - squares split 5/8 ACT (Square activation, accum_out f32) and 3/8 DVE
    (tensor_tensor mult at 2/cycle bf16) whose chunk sums reduce on the PE
    via a ones^T matmul into one PSUM bank (DVE tensor_reduce is too slow
    to hide; tensor_tensor_reduce crashes this lowering; ACT/DVE ops on 3D
    APs with >8192 free elements crash or silently corrupt — keep APs 2D)
  - PE: s_k via one-hot^T @ feat bf16 matmuls accumulating in PSUM f32
  - tiny [3, D] + [128, 4] + [1, 512] outputs; final combine on host in f64.
Measured on axon trn2 (steady-state rep-diff): ~38.2 us/pass/core, 2.4x
over the f32 baseline; correctness rel err 3.7e-4. Attempted and rejected:
sub-block ACT->DVE square rebalance via fp8-input tensor_tensor (regressed
to 42.4 — fp8-in TT is slower than 1x and adds a third consumer to the fp8
tiles), f32-out DVE reduce (tensor_reduce is 1x for f32 AND bf16), and PE
DoubleRow for the one-hot matmuls (needs element-interleaved rhs, which
neither DMA descriptors nor spare DVE cycles can provide).
"""

import sys

if "/opt/trn_rl_repo" not in sys.path:
    sys.path.insert(0, "/opt/trn_rl_repo")

import numpy as np

import concourse.bacc as bacc
import concourse.tile as tile
from concourse import mybir
from concourse.bass_utils import run_bass_kernel_spmd

B = 32768
D = 2048
NCLS = 3
NCORES = 8
ROWS = B // NCORES      # 4096 rows per core
P = 128                 # partitions
BLOCKS = ROWS // P      # 32 row-blocks of 128
G = 8                   # row-blocks per supertile (one DMA: 2 MiB HBM-side)
ST = BLOCKS // G        # 4 supertiles
NJ = D // 512           # 4 column chunks of 512 (one PSUM bank each)
NA = 5                  # fp8-resident row-blocks per supertile; rest bf16
SPILL = 0               # cols of the fp8 share squared on DVE instead of
                        # ACT. Measured: SPILL=1024 REGRESSES (42.4 vs 38.8
                        # us) — DVE tensor_tensor on fp8 input runs slower
                        # than 1x and/or the extra ft8 consumer stalls
                        # buffer recycling. Keep 0: ACT squares the whole
                        # fp8 share.

# feat lives in HBM as fp8-e4m3 (host casts f32->e4m3 round-to-nearest
# inside kernel(), quartering HBM traffic vs f32) and is upcast EXACTLY to
# bf16 by the SWDGE DMA cast on the way into SBUF. On-chip compute dtype is
# bf16; all accumulation is f32 on-chip / f64 on host. e4m3 quantization
# biases sum(feat^2) by E[delta^2] ~ +1.3e-3 — ~15x inside the tolerance.
STREAM_DT = mybir.dt.float8e4
MM_DT = mybir.dt.bfloat16

_NC_CACHE = {}


def _build_nc(mm_dt=MM_DT, reps=1, dma_engines=("sync",), g=G, bufs=4,
              na=NA):
    """reps>1 repeats the whole feat pass inside one NEFF (identical outputs
    each rep) — used only for wall-clock benchmarking where the per-dispatch
    overhead (~80 ms over axon) must be amortized away."""
    st_count = BLOCKS // g
    nc = bacc.Bacc("TRN2", target_bir_lowering=False, debug=False)

    feat_in = nc.dram_tensor("feat", [ROWS, D], STREAM_DT, kind="ExternalInput")
    oh_in = nc.dram_tensor(
        "onehot", [P, BLOCKS * NCLS], STREAM_DT, kind="ExternalInput"
    )
    s_out = nc.dram_tensor("csum", [NCLS, D], mybir.dt.float32, kind="ExternalOutput")
    qa_out = nc.dram_tensor(
        "sqsum_a", [P, st_count], mybir.dt.float32, kind="ExternalOutput"
    )
    qd_out = nc.dram_tensor(
        "sqsum_d", [1, 512], mybir.dt.float32, kind="ExternalOutput"
    )
    qv_out = nc.dram_tensor(
        "sqsum_v", [P, 2 * st_count], mybir.dt.float32, kind="ExternalOutput"
    )

    # [ROWS, D] -> [ST, P, G, D]: supertile st, partition p holds G rows
    # (one from each of its G row-blocks), 4 KB contiguous per row.
    featv = feat_in.ap().rearrange("(s n p) d -> s p n d", p=P, n=g)

    with tile.TileContext(nc) as tc:
        with (
            tc.tile_pool(name="consts", bufs=1) as consts,
            tc.tile_pool(name="feat8", bufs=bufs) as fpool8,
            tc.tile_pool(name="featb", bufs=bufs) as fpoolb,
            tc.tile_pool(name="scra", bufs=1) as sapool,
            tc.tile_pool(name="scrd", bufs=3) as sdpool,
            tc.tile_pool(name="outs", bufs=1) as opool,
            tc.tile_pool(name="psum", bufs=1, space="PSUM") as ppool,
        ):
            # One fp8 one-hot shipped from HBM plus a bf16 copy via SWDGE
            # cast (0/1 exact in both): fp8 blocks use oh8, bf16 blocks ohb.
            oh8 = consts.tile([P, BLOCKS * NCLS], STREAM_DT)
            nc.gpsimd.dma_start(out=oh8, in_=oh_in.ap())
            ohb = consts.tile([P, BLOCKS * NCLS], mm_dt)
            nc.gpsimd.dma_start(out=ohb, in_=oh_in.ap())
            ones = consts.tile([P, 1], mm_dt)
            nc.vector.memset(ones, 1.0)

            # PE warm-up: absorb the onehot-DMA wait into a throwaway matmul
            # so real matmuls carry only their feat-DMA wait (the lowered
            # LDWEIGHTS struct holds a single sync-wait slot).
            warm = ppool.tile([NCLS, 1], mybir.dt.float32, name="warm", tag="warm")
            nc.tensor.matmul(warm, oh8[:, 0:NCLS], oh8[:, 0:1], start=True, stop=True)

            acc_a = opool.tile([P, st_count], mybir.dt.float32)
            acc_v = opool.tile([P, 2 * st_count], mybir.dt.float32)
            nc.vector.memset(acc_v, 0.0)
            # elementwise outputs we never read; only accum_out matters.
            # NB: keep all large-free-size APs 2D — ACT/DVE instructions over
            # 3D APs with >8192 free elements crash or silently corrupt on
            # this toolchain (bisected; [128,16384] 2D is fine).
            acols = na * D - SPILL
            sq_a = sapool.tile([P, acols], mybir.dt.bfloat16)
            sq_8 = sapool.tile([P, max(SPILL, 1)], mybir.dt.bfloat16)
            psums = [
                ppool.tile(
                    [NCLS, 512], mybir.dt.float32, name=f"ps{j}", tag=f"ps{j}"
                )
                for j in range(NJ)
            ]
            # DVE-share squares: most 512-chunks reduce over partitions on
            # the PE (ones^T matmul into one [1,512] bank); the last D
            # columns reduce on DVE itself (tensor_reduce, 1x) so PE stays
            # balanced with ACT.
            nd = (g - na) * D
            nsq = max(nd - D, 0) // 512
            if na < g:
                ps_s = ppool.tile([1, 512], mybir.dt.float32, name="ps_s", tag="ps_s")

            for _rep in range(reps):
                for st in range(st_count):
                    # ACT's share stays fp8 in SBUF — those elements cross
                    # the SBUF AXI ports (the binding fabric resource) at
                    # 1 B/elem instead of 2. Same SWDGE queue as the cast
                    # stream: a second concurrent queue (sync HWDGE) was
                    # measured to serialize against it AND cost ~7 us/pass
                    # of lost overlap vs the single-queue structure.
                    ft8 = fpool8.tile([P, na, D], STREAM_DT, name="ft8")
                    nc.gpsimd.dma_start(out=ft8, in_=featv[st][:, 0:na, :])
                    # DVE/PE share: SWDGE casting DMA fp8->bf16 (exact)
                    ftb = fpoolb.tile([P, g - na, D], mm_dt, name="ftb")
                    nc.gpsimd.dma_start(out=ftb, in_=featv[st][:, na:g, :])

                    f8 = ft8.rearrange("p n d -> p (n d)")
                    fb = ftb.rearrange("p n d -> p (n d)")
                    nc.scalar.activation(
                        out=sq_a,
                        in_=f8[:, 0:acols],
                        func=mybir.ActivationFunctionType.Square,
                        accum_out=acc_a[:, st : st + 1],
                    )
                    if SPILL > 0:
                        # fp8 spill squared on DVE (TT 1x on fp8 in, bf16
                        # out) + DVE free-dim reduce; keeps ACT at ~8us/st
                        nc.vector.tensor_tensor(
                            out=sq_8,
                            in0=f8[:, acols : na * D],
                            in1=f8[:, acols : na * D],
                            op=mybir.AluOpType.mult,
                        )
                        nc.vector.tensor_reduce(
                            out=acc_v[:, st_count + st : st_count + st + 1],
                            in_=sq_8,
                            axis=mybir.AxisListType.X,
                            op=mybir.AluOpType.add,
                        )
                    if na < g:
                        # DVE: square only (tensor_tensor mult, 2/cyc bf16);
                        # tensor_tensor_reduce crashes in this lowering.
                        sq_d = sdpool.tile([P, nd], mybir.dt.bfloat16, name="sqd")
                        nc.vector.tensor_tensor(
                            out=sq_d,
                            in0=fb,
                            in1=fb,
                            op=mybir.AluOpType.mult,
                        )
                        for j in range(nsq):
                            nc.tensor.matmul(
                                ps_s,
                                ones,
                                sq_d[:, j * 512 : (j + 1) * 512],
                                start=(st == 0 and j == 0),
                                stop=(st == st_count - 1 and j == nsq - 1),
                            )
                        if nsq * 512 < nd:
                            nc.vector.tensor_reduce(
                                out=acc_v[:, st : st + 1],
                                in_=sq_d[:, nsq * 512 : nd],
                                axis=mybir.AxisListType.X,
                                op=mybir.AluOpType.add,
                            )

                    for n in range(g):
                        blk = st * g + n
                        if n < na:
                            lhsT = oh8[:, blk * NCLS : (blk + 1) * NCLS]
                            src, ni = ft8, n
                        else:
                            lhsT = ohb[:, blk * NCLS : (blk + 1) * NCLS]
                            src, ni = ftb, n - na
                        for j in range(NJ):
                            nc.tensor.matmul(
                                psums[j],
                                lhsT,
                                src[:, ni, j * 512 : (j + 1) * 512],
                                start=(blk == 0),
                                stop=(blk == BLOCKS - 1),
                            )

            s_sb = opool.tile([NCLS, D], mybir.dt.float32)
            # keep the warm-up matmul alive (its result is overwritten by the
            # ps0 copy below before anything reads s_sb)
            nc.vector.tensor_copy(s_sb[:, 0:1], warm)
            for j in range(NJ):
                nc.vector.tensor_copy(s_sb[:, j * 512 : (j + 1) * 512], psums[j])
            q_sb = opool.tile([1, 512], mybir.dt.float32)
            if na < g:
                nc.vector.tensor_copy(q_sb, ps_s)
            else:
                nc.vector.memset(q_sb, 0.0)
            nc.sync.dma_start(out=s_out.ap(), in_=s_sb)
            nc.sync.dma_start(out=qa_out.ap(), in_=acc_a)
            nc.sync.dma_start(out=qd_out.ap(), in_=q_sb)
            nc.sync.dma_start(out=qv_out.ap(), in_=acc_v)

    nc.compile()
    return nc


def _get_nc(mm_dt=MM_DT):
    key = str(mm_dt)
    if key not in _NC_CACHE:
        _NC_CACHE[key] = _build_nc(mm_dt)
    return _NC_CACHE[key]


def _one_hot_t(ls, np_dt=None):
    """[ROWS] int labels -> [P, BLOCKS*NCLS] in SBUF layout:
    row p, cols [blk*3 : blk*3+3] = one-hot of label[blk*128 + p]."""
    if np_dt is None:
        np_dt = mybir.dt.np(STREAM_DT)
    oh = np.zeros((BLOCKS, P, NCLS), np_dt)
    idx = ls.reshape(BLOCKS, P)
    oh[np.arange(BLOCKS)[:, None], np.arange(P)[None, :], idx] = 1.0
    return np.ascontiguousarray(oh.transpose(1, 0, 2).reshape(P, BLOCKS * NCLS))


def _feat_maps(feat, label, mm_dt=MM_DT):
    """Full f32 feat + labels -> per-core in_maps with feat cast to the
    stream dtype (round-to-nearest via ml_dtypes)."""
    np_st = mybir.dt.np(STREAM_DT)
    feat_c = np.ascontiguousarray(np.asarray(feat), dtype=np.float32).astype(np_st)
    in_maps = []
    for c in range(NCORES):
        in_maps.append(
            {
                "feat": feat_c[c * ROWS : (c + 1) * ROWS],
                "onehot": _one_hot_t(label[c * ROWS : (c + 1) * ROWS], np_st),
            }
        )
    return in_maps


def _run(feat, label, centers, trace=False, mm_dt=MM_DT):
    label = np.asarray(label).astype(np.int32).ravel()
    centers = np.asarray(centers, dtype=np.float32)
    assert np.asarray(feat).shape == (B, D) and label.shape == (B,)

    nc = _get_nc(mm_dt)
    in_maps = _feat_maps(feat, label, mm_dt)
    res = run_bass_kernel_spmd(
        nc, in_maps, core_ids=list(range(NCORES)), trace=trace
    )

    s_tot = np.zeros((NCLS, D), np.float64)
    S_ff = 0.0
    for r in res.results:
        s_tot += r["csum"].astype(np.float64)
        S_ff += float(r["sqsum_a"].astype(np.float64).sum())
        S_ff += float(r["sqsum_d"].astype(np.float64).sum())  # [1,512] col sums
        S_ff += float(r["sqsum_v"].astype(np.float64).sum())  # DVE self-reduced

    n_k = np.bincount(label, minlength=NCLS).astype(np.float64)
    c64 = centers.astype(np.float64)
    cn_k = np.sum(c64 * c64, axis=1)          # ||c_k||^2
    T1 = float(np.sum(s_tot * c64))
    C1 = float(np.sum(n_k * cn_k))
    main = S_ff - 2.0 * T1 + C1
    T_all = float(np.dot(s_tot.sum(axis=0), c64.sum(axis=0)))
    total = 3.0 * S_ff - 2.0 * T_all + B * float(np.sum(cn_k))
    distocen = total - main
    loss = main * (1.0 + 1.0 / distocen) / 2.0 / B
    return np.asarray(loss, dtype=np.float32), res


def kernel(feat, label, centers):
    # Best-effort retry: a previous crashed NEFF on the shared device can
    # leave a core in NRT_EXEC_UNIT_UNRECOVERABLE; a short wait and a fresh
    # attempt usually recovers it.
    import time

    last = None
    for attempt in range(3):
        try:
            loss, _ = _run(feat, label, centers, trace=False)
            return loss
        except Exception as e:  # jax.errors.JaxRuntimeError and friends
            last = e
            if attempt < 2:
                time.sleep(30 * (attempt + 1))
    raise last


# revision 48
# speedup vs baseline: 1.1657x; 1.1657x over previous
"""CenterLossA on 8 Trainium2 NeuronCores — fp8-streamed, bf16 compute.

loss = main * (1 + 1/distocen) / 2 / B, where
  main     = sum_i ||f_i - c_{l_i}||^2
  distocen = sum_i sum_{k != l_i} ||f_i - c_k||^2

Algebraic reduction (everything needed from feat in ONE pass):
  main  = S_ff - 2*T1 + C1
  total = sum_i sum_k ||f_i - c_k||^2 = 3*S_ff - 2*T_all + B*Cn
  distocen = total - main
with
  S_ff  = sum(feat^2)                      (scalar)
  s_k   = sum_{i: l_i = k} f_i             ([3, D] per-class column sums)
  T1    = sum_k s_k . c_k ;  T_all = (sum_k s_k) . (sum_k c_k)
  C1    = sum_k n_k ||c_k||^2 ;  Cn = sum_k ||c_k||^2 ; n_k = count(label==k)

This is memory-regime: the f32 version sits exactly on the per-core HBM
streaming floor (~96-98 us for 32 MiB at ~350 GB/s/core; DMA-only programs
measure the same), so the only levers are BYTES and then engine throughput:
  f32 stream                 : ~92-98 us/pass  (HBM-bound, 32 MiB)
  bf16 stream                : ~48 us/pass     (HBM-bound, 16 MiB)
  fp8 stream, all cast->bf16 : ~42 us/pass     (SBUF-fabric-bound: 16 MiB of
                                                bf16 written at ~407 GB/s)
  fp8 mixed residency        : ~39 us/pass     (ACT-bound: ACT's 5/8 share
                                                stays fp8 in SBUF, only
                                                DVE's 3/8 is cast to bf16)
Engine poles at the final split: ACT 35.8, PE 34.1 (27.3 one-hot matmuls +
6.8 square-chunk reduce), DMA ~30 single-queue, DVE 22. All feat DMAs ride
ONE SWDGE queue: a second concurrent queue (sync HWDGE fp8 + gpsimd cast)
was measured to serialize against it and add ~7 us/pass of lost overlap.
ACT squares fp8 at full 1x rate (measured; no fp8 penalty).

Numerics: host casts feat f32 -> e4m3 (RN); the SWDGE DMA upcasts e4m3 ->
bf16 EXACTLY. e4m3 quantization biases S_ff by E[delta^2] ~ 1e-3; measured
end-to-end loss error 3.7e-4 vs the f32 reference — ~50x inside the 2e-2
gate. All accumulation is f32 on-chip / f64 on host.

Device kernel (data-parallel over batch, 4096 rows/core), one stream:
  - SWDGE casting DMA: 4 supertiles of [128, 8x2048] (2 MiB fp8 HBM-side ->
    4 MiB bf16 SBUF-side), triple-buffered
  - squares split 5/8 ACT (Square activation, accum_out f32) and 3/8 DVE
    (tensor_tensor mult at 2/cycle bf16) whose chunk sums reduce on the PE
    via a ones^T matmul into one PSUM bank (DVE tensor_reduce is too slow
    to hide; tensor_tensor_reduce crashes this lowering; ACT/DVE ops on 3D
    APs with >8192 free elements crash or silently corrupt — keep APs 2D)
  - PE: s_k via one-hot^T @ feat bf16 matmuls accumulating in PSUM f32
  - tiny [3, D] + [128, 4] + [1, 512] outputs; final combine on host in f64.
Measured on axon trn2 (steady-state rep-diff): ~38.2 us/pass/core, 2.4x
over the f32 baseline; correctness rel err 3.7e-4. Attempted and rejected:
sub-block ACT->DVE square rebalance via fp8-input tensor_tensor (regressed
to 42.4 — fp8-in TT is slower than 1x and adds a third consumer to the fp8
tiles), f32-out DVE reduce (tensor_reduce is 1x for f32 AND bf16), and PE
DoubleRow for the one-hot matmuls (needs element-interleaved rhs, which
neither DMA descriptors nor spare DVE cycles can provide).
"""

import sys

if "/opt/trn_rl_repo" not in sys.path:
    sys.path.insert(0, "/opt/trn_rl_repo")

import numpy as np

import concourse.bacc as bacc
import concourse.tile as tile
from concourse import mybir
from concourse.bass_utils import run_bass_kernel_spmd

B = 32768
D = 2048
NCLS = 3
NCORES = 8
ROWS = B // NCORES      # 4096 rows per core
P = 128                 # partitions
BLOCKS = ROWS // P      # 32 row-blocks of 128
G = 8                   # row-blocks per supertile (one DMA: 2 MiB HBM-side)
ST = BLOCKS // G        # 4 supertiles
NJ = D // 512           # 4 column chunks of 512 (one PSUM bank each)
NA = 5                  # fp8-resident row-blocks per supertile; rest bf16
SPILL = 0               # cols of the fp8 share squared on DVE instead of
                        # ACT. Measured: SPILL=1024 REGRESSES (42.4 vs 38.8
                        # us) — DVE tensor_tensor on fp8 input runs slower
                        # than 1x and/or the extra ft8 consumer stalls
                        # buffer recycling. Keep 0: ACT squares the whole
                        # fp8 share.

# feat lives in HBM as fp8-e4m3 (host casts f32->e4m3 round-to-nearest
# inside kernel(), quartering HBM traffic vs f32) and is upcast EXACTLY to
# bf16 by the SWDGE DMA cast on the way into SBUF. On-chip compute dtype is
# bf16; all accumulation is f32 on-chip / f64 on host. e4m3 quantization
# biases sum(feat^2) by E[delta^2] ~ +1.3e-3 — ~15x inside the tolerance.
STREAM_DT = mybir.dt.float8e4
MM_DT = mybir.dt.bfloat16

_NC_CACHE = {}


def _build_nc(mm_dt=MM_DT, reps=1, dma_engines=("sync",), g=G, bufs=3,
              na=NA):
    # bufs=3 / sq_d bufs=2 are measured optima: bufs=4 + sq_d bufs=3
    # REGRESSED to 44.2 us (vs 38.2) — deeper pools hurt this pipeline.
    """reps>1 repeats the whole feat pass inside one NEFF (identical outputs
    each rep) — used only for wall-clock benchmarking where the per-dispatch
    overhead (~80 ms over axon) must be amortized away."""
    st_count = BLOCKS // g
    nc = bacc.Bacc("TRN2", target_bir_lowering=False, debug=False)

    feat_in = nc.dram_tensor("feat", [ROWS, D], STREAM_DT, kind="ExternalInput")
    oh_in = nc.dram_tensor(
        "onehot", [P, BLOCKS * NCLS], STREAM_DT, kind="ExternalInput"
    )
    s_out = nc.dram_tensor("csum", [NCLS, D], mybir.dt.float32, kind="ExternalOutput")
    qa_out = nc.dram_tensor(
        "sqsum_a", [P, st_count], mybir.dt.float32, kind="ExternalOutput"
    )
    qd_out = nc.dram_tensor(
        "sqsum_d", [1, 512], mybir.dt.float32, kind="ExternalOutput"
    )
    qv_out = nc.dram_tensor(
        "sqsum_v", [P, 2 * st_count], mybir.dt.float32, kind="ExternalOutput"
    )

    # [ROWS, D] -> [ST, P, G, D]: supertile st, partition p holds G rows
    # (one from each of its G row-blocks), 4 KB contiguous per row.
    featv = feat_in.ap().rearrange("(s n p) d -> s p n d", p=P, n=g)

    with tile.TileContext(nc) as tc:
        with (
            tc.tile_pool(name="consts", bufs=1) as consts,
            tc.tile_pool(name="feat8", bufs=bufs) as fpool8,
            tc.tile_pool(name="featb", bufs=bufs) as fpoolb,
            tc.tile_pool(name="scra", bufs=1) as sapool,
            tc.tile_pool(name="scrd", bufs=2) as sdpool,
            tc.tile_pool(name="outs", bufs=1) as opool,
            tc.tile_pool(name="psum", bufs=1, space="PSUM") as ppool,
        ):
            # One fp8 one-hot shipped from HBM plus a bf16 copy via SWDGE
            # cast (0/1 exact in both): fp8 blocks use oh8, bf16 blocks ohb.
            oh8 = consts.tile([P, BLOCKS * NCLS], STREAM_DT)
            nc.gpsimd.dma_start(out=oh8, in_=oh_in.ap())
            ohb = consts.tile([P, BLOCKS * NCLS], mm_dt)
            nc.gpsimd.dma_start(out=ohb, in_=oh_in.ap())
            ones = consts.tile([P, 1], mm_dt)
            nc.vector.memset(ones, 1.0)

            # PE warm-up: absorb the onehot-DMA wait into a throwaway matmul
            # so real matmuls carry only their feat-DMA wait (the lowered
            # LDWEIGHTS struct holds a single sync-wait slot).
            warm = ppool.tile([NCLS, 1], mybir.dt.float32, name="warm", tag="warm")
            nc.tensor.matmul(warm, oh8[:, 0:NCLS], oh8[:, 0:1], start=True, stop=True)

            acc_a = opool.tile([P, st_count], mybir.dt.float32)
            acc_v = opool.tile([P, 2 * st_count], mybir.dt.float32)
            nc.vector.memset(acc_v, 0.0)
            # elementwise outputs we never read; only accum_out matters.
            # NB: keep all large-free-size APs 2D — ACT/DVE instructions over
            # 3D APs with >8192 free elements crash or silently corrupt on
            # this toolchain (bisected; [128,16384] 2D is fine).
            acols = na * D - SPILL
            sq_a = sapool.tile([P, acols], mybir.dt.bfloat16)
            sq_8 = sapool.tile([P, max(SPILL, 1)], mybir.dt.bfloat16)
            psums = [
                ppool.tile(
                    [NCLS, 512], mybir.dt.float32, name=f"ps{j}", tag=f"ps{j}"
                )
                for j in range(NJ)
            ]
            # DVE-share squares: most 512-chunks reduce over partitions on
            # the PE (ones^T matmul into one [1,512] bank); the last D
            # columns reduce on DVE itself (tensor_reduce, 1x) so PE stays
            # balanced with ACT.
            nd = (g - na) * D
            nsq = max(nd - D, 0) // 512
            if na < g:
                ps_s = ppool.tile([1, 512], mybir.dt.float32, name="ps_s", tag="ps_s")

            for _rep in range(reps):
                for st in range(st_count):
                    # ACT's share stays fp8 in SBUF — those elements cross
                    # the SBUF AXI ports (the binding fabric resource) at
                    # 1 B/elem instead of 2. Same SWDGE queue as the cast
                    # stream: a second concurrent queue (sync HWDGE) was
                    # measured to serialize against it AND cost ~7 us/pass
                    # of lost overlap vs the single-queue structure.
                    ft8 = fpool8.tile([P, na, D], STREAM_DT, name="ft8")
                    nc.gpsimd.dma_start(out=ft8, in_=featv[st][:, 0:na, :])
                    # DVE/PE share: SWDGE casting DMA fp8->bf16 (exact)
                    ftb = fpoolb.tile([P, g - na, D], mm_dt, name="ftb")
                    nc.gpsimd.dma_start(out=ftb, in_=featv[st][:, na:g, :])

                    f8 = ft8.rearrange("p n d -> p (n d)")
                    fb = ftb.rearrange("p n d -> p (n d)")
                    nc.scalar.activation(
                        out=sq_a,
                        in_=f8[:, 0:acols],
                        func=mybir.ActivationFunctionType.Square,
                        accum_out=acc_a[:, st : st + 1],
                    )
                    if SPILL > 0:
                        # fp8 spill squared on DVE (TT 1x on fp8 in, bf16
                        # out) + DVE free-dim reduce; keeps ACT at ~8us/st
                        nc.vector.tensor_tensor(
                            out=sq_8,
                            in0=f8[:, acols : na * D],
                            in1=f8[:, acols : na * D],
                            op=mybir.AluOpType.mult,
                        )
                        nc.vector.tensor_reduce(
                            out=acc_v[:, st_count + st : st_count + st + 1],
                            in_=sq_8,
                            axis=mybir.AxisListType.X,
                            op=mybir.AluOpType.add,
                        )
                    if na < g:
                        # DVE: square only (tensor_tensor mult, 2/cyc bf16);
                        # tensor_tensor_reduce crashes in this lowering.
                        sq_d = sdpool.tile([P, nd], mybir.dt.bfloat16, name="sqd")
                        nc.vector.tensor_tensor(
                            out=sq_d,
                            in0=fb,
                            in1=fb,
                            op=mybir.AluOpType.mult,
                        )
                        for j in range(nsq):
                            nc.tensor.matmul(
                                ps_s,
                                ones,
                                sq_d[:, j * 512 : (j + 1) * 512],
                                start=(st == 0 and j == 0),
                                stop=(st == st_count - 1 and j == nsq - 1),
                            )
                        if nsq * 512 < nd:
                            nc.vector.tensor_reduce(
                                out=acc_v[:, st : st + 1],
                                in_=sq_d[:, nsq * 512 : nd],
                                axis=mybir.AxisListType.X,
                                op=mybir.AluOpType.add,
                            )

                    for n in range(g):
                        blk = st * g + n
                        if n < na:
                            lhsT = oh8[:, blk * NCLS : (blk + 1) * NCLS]
                            src, ni = ft8, n
                        else:
                            lhsT = ohb[:, blk * NCLS : (blk + 1) * NCLS]
                            src, ni = ftb, n - na
                        for j in range(NJ):
                            nc.tensor.matmul(
                                psums[j],
                                lhsT,
                                src[:, ni, j * 512 : (j + 1) * 512],
                                start=(blk == 0),
                                stop=(blk == BLOCKS - 1),
                            )

            s_sb = opool.tile([NCLS, D], mybir.dt.float32)
            # keep the warm-up matmul alive (its result is overwritten by the
            # ps0 copy below before anything reads s_sb)
            nc.vector.tensor_copy(s_sb[:, 0:1], warm)
            for j in range(NJ):
                nc.vector.tensor_copy(s_sb[:, j * 512 : (j + 1) * 512], psums[j])
            q_sb = opool.tile([1, 512], mybir.dt.float32)
            if na < g:
                nc.vector.tensor_copy(q_sb, ps_s)
            else:
                nc.vector.memset(q_sb, 0.0)
            nc.sync.dma_start(out=s_out.ap(), in_=s_sb)
            nc.sync.dma_start(out=qa_out.ap(), in_=acc_a)
            nc.sync.dma_start(out=qd_out.ap(), in_=q_sb)
            nc.sync.dma_start(out=qv_out.ap(), in_=acc_v)

    nc.compile()
    return nc


def _get_nc(mm_dt=MM_DT):
    key = str(mm_dt)
    if key not in _NC_CACHE:
        _NC_CACHE[key] = _build_nc(mm_dt)
    return _NC_CACHE[key]


def _one_hot_t(ls, np_dt=None):
    """[ROWS] int labels -> [P, BLOCKS*NCLS] in SBUF layout:
    row p, cols [blk*3 : blk*3+3] = one-hot of label[blk*128 + p]."""
    if np_dt is None:
        np_dt = mybir.dt.np(STREAM_DT)
    oh = np.zeros((BLOCKS, P, NCLS), np_dt)
    idx = ls.reshape(BLOCKS, P)
    oh[np.arange(BLOCKS)[:, None], np.arange(P)[None, :], idx] = 1.0
    return np.ascontiguousarray(oh.transpose(1, 0, 2).reshape(P, BLOCKS * NCLS))


def _feat_maps(feat, label, mm_dt=MM_DT):
    """Full f32 feat + labels -> per-core in_maps with feat cast to the
    stream dtype (round-to-nearest via ml_dtypes)."""
    np_st = mybir.dt.np(STREAM_DT)
    feat_c = np.ascontiguousarray(np.asarray(feat), dtype=np.float32).astype(np_st)
    in_maps = []
    for c in range(NCORES):
        in_maps.append(
            {
                "feat": feat_c[c * ROWS : (c + 1) * ROWS],
                "onehot": _one_hot_t(label[c * ROWS : (c + 1) * ROWS], np_st),
            }
        )
    return in_maps


def _run(feat, label, centers, trace=False, mm_dt=MM_DT):
    label = np.asarray(label).astype(np.int32).ravel()
    centers = np.asarray(centers, dtype=np.float32)
    assert np.asarray(feat).shape == (B, D) and label.shape == (B,)

    nc = _get_nc(mm_dt)
    in_maps = _feat_maps(feat, label, mm_dt)
    res = run_bass_kernel_spmd(
        nc, in_maps, core_ids=list(range(NCORES)), trace=trace
    )

    s_tot = np.zeros((NCLS, D), np.float64)
    S_ff = 0.0
    for r in res.results:
        s_tot += r["csum"].astype(np.float64)
        S_ff += float(r["sqsum_a"].astype(np.float64).sum())
        S_ff += float(r["sqsum_d"].astype(np.float64).sum())  # [1,512] col sums
        S_ff += float(r["sqsum_v"].astype(np.float64).sum())  # DVE self-reduced

    n_k = np.bincount(label, minlength=NCLS).astype(np.float64)
    c64 = centers.astype(np.float64)
    cn_k = np.sum(c64 * c64, axis=1)          # ||c_k||^2
    T1 = float(np.sum(s_tot * c64))
    C1 = float(np.sum(n_k * cn_k))
    main = S_ff - 2.0 * T1 + C1
    T_all = float(np.dot(s_tot.sum(axis=0), c64.sum(axis=0)))
    total = 3.0 * S_ff - 2.0 * T_all + B * float(np.sum(cn_k))
    distocen = total - main
    loss = main * (1.0 + 1.0 / distocen) / 2.0 / B
    return np.asarray(loss, dtype=np.float32), res


def kernel(feat, label, centers):
    # Best-effort retry: a previous crashed NEFF on the shared device can
    # leave a core in NRT_EXEC_UNIT_UNRECOVERABLE; a short wait and a fresh
    # attempt usually recovers it.
    import time

    last = None
    for attempt in range(3):
        try:
            loss, _ = _run(feat, label, centers, trace=False)
            return loss
        except Exception as e:  # jax.errors.JaxRuntimeError and friends
            last = e
            if attempt < 2:
                time.sleep(30 * (attempt + 1))
    raise last
